# revision 1
# baseline (speedup 1.0000x reference)
"""Trainium2 Bass kernel for GQA attention (B=2, S=2048, D=2048, H=16, G=4 kv-heads,
DH=128) with interleaved RoPE (base 1e6) and causal mask.

Sharding: one (batch b, kv-group g) pair per NeuronCore -> 8 cores. Each core
computes its 4 q-heads against its single kv-head (Megatron-style column-split
of w_q/w_k/w_v, row-split of w_o) and produces a partial (S, D) output-projection
product; the host sums the 4 partials per batch and adds bo.

Device dataflow per core (all matmuls bf16 with f32 PSUM accumulate):
  A) qT/kT/vT = W^T-slices @ x^T (transposed projections, dmodel contraction),
     interleaved RoPE applied in the transposed layout via a +-1 permutation
     matmul plus two DVE multiplies with host-provided cos/sin tables;
     v transposed back to [sk, dh] via PE transpose.
  B) per (head, sq-chunk of 512): scoresT tiles [sk=128, sq=512] via PE,
     exp on ScalarE (scale=1/sqrt(128)) straight out of PSUM -> bf16 attn
     weights, causal masking on diagonal tiles, DVE accumulates exp sums,
     PV matmul accumulates out^T [dh, sq] in PSUM over sk tiles; column sums
     via a ones-vector matmul, reciprocal + partition-broadcast, fused
     normalize-and-evict on DVE.
  C) partial = out_heads^T^T @ wo^T-slice, streamed to DRAM as f32.
"""
import sys
import os

if '/opt/trn_rl_repo' not in sys.path:
    sys.path.insert(0, '/opt/trn_rl_repo')

import numpy as np
import ml_dtypes

from contextlib import ExitStack

import concourse.bass as bass
import concourse.mybir as mybir
import concourse.tile as tile
from concourse import bacc
import concourse.bass_utils as bass_utils
from concourse.masks import make_identity

BF = mybir.dt.bfloat16
F32 = mybir.dt.float32
AF = mybir.ActivationFunctionType
ALU = mybir.AluOpType

B, S, D, H, G = 2, 2048, 2048, 16, 4
DH = 128
HPC = H // G          # q heads per core
KT = D // 128         # dmodel k-tiles
NCH = S // 512        # sq chunks
SCALE = float(1.0 / np.sqrt(DH))
N_CORES = 8

TRACE = False          # set by test harness to capture an NTFF profile
LAST_RESULTS = None    # BassKernelResults of the most recent run (for test.py)

_PROGRAM = None


def _build_program():
    nc = bacc.Bacc("TRN2", target_bir_lowering=False, debug=False,
                   num_devices=N_CORES)

    def din(name, shape, dtype=BF):
        return nc.dram_tensor(name, shape, dtype, kind="ExternalInput").ap()

    xT_d = din("xT", [D, S])
    wq_d = din("wqT", [D, 512])
    wk_d = din("wkT", [D, DH])
    wv_d = din("wvT", [D, DH])
    wo_d = din("woT", [512, D])
    cos_d = din("cosT", [DH, S], F32)
    sin_d = din("sinT", [DH, S], F32)
    perm_d = din("permT", [DH, DH])
    mask_d = din("maskLT", [DH, DH])
    bq_d = din("bq", [DH, HPC], F32)
    bk_d = din("bk", [DH, 1], F32)
    bv_d = din("bv", [DH, 1], F32)
    out_d = nc.dram_tensor("part", [S, D], F32, kind="ExternalOutput").ap()

    with tile.TileContext(nc) as tc, ExitStack() as ctx:
        consts = ctx.enter_context(tc.tile_pool(name="consts", bufs=1))

        # persistent SBUF tensors
        wq_sb = consts.tile([128, KT, 512], BF, tag="wq")
        wk_sb = consts.tile([128, KT, DH], BF, tag="wk")
        wv_sb = consts.tile([128, KT, DH], BF, tag="wv")
        wo_sb = consts.tile([128, HPC, D], BF, tag="wo")
        mask_sb = consts.tile([128, 128], BF, tag="mask")
        bq_sb = consts.tile([128, HPC], F32, tag="bq")
        bk_sb = consts.tile([128, 1], F32, tag="bk")
        bv_sb = consts.tile([128, 1], F32, tag="bv")
        ones_sb = consts.tile([128, 1], F32, tag="ones")
        qT_sb = consts.tile([128, HPC, S], BF, tag="qT")
        kT_sb = consts.tile([128, S], BF, tag="kT")
        v_sb = consts.tile([128, KT, DH], BF, tag="v")
        outT_sb = consts.tile([128, HPC, S], BF, tag="outT")

        for kk in range(KT):
            nc.sync.dma_start(out=wq_sb[:, kk, :], in_=wq_d[kk * 128:(kk + 1) * 128, :])
            nc.sync.dma_start(out=wk_sb[:, kk, :], in_=wk_d[kk * 128:(kk + 1) * 128, :])
            nc.sync.dma_start(out=wv_sb[:, kk, :], in_=wv_d[kk * 128:(kk + 1) * 128, :])
        for h in range(HPC):
            nc.sync.dma_start(out=wo_sb[:, h, :], in_=wo_d[h * 128:(h + 1) * 128, :])
        nc.sync.dma_start(out=mask_sb, in_=mask_d)
        nc.sync.dma_start(out=bq_sb, in_=bq_d)
        nc.sync.dma_start(out=bk_sb, in_=bk_d)
        nc.sync.dma_start(out=bv_sb, in_=bv_d)
        nc.vector.memset(ones_sb, 1.0)

        # ---------------- Phase A: projections + RoPE + v transpose ----------
        with tc.tile_pool(name="aconst", bufs=1) as aconst, \
             tc.tile_pool(name="pps", bufs=2, space="PSUM") as pps, \
             tc.tile_pool(name="rps", bufs=2, space="PSUM") as rps, \
             tc.tile_pool(name="vtp", bufs=2, space="PSUM") as vtp, \
             tc.tile_pool(name="rawp", bufs=3) as rawp, \
             tc.tile_pool(name="tmpp", bufs=3) as tmpp:

            xT_sb = aconst.tile([128, KT, S], BF, tag="xT")
            cos_sb = aconst.tile([128, S], F32, tag="cos")
            sin_sb = aconst.tile([128, S], F32, tag="sin")
            perm_sb = aconst.tile([128, 128], BF, tag="perm")
            ident_sb = aconst.tile([128, 128], F32, tag="ident")

            for kk in range(KT):
                nc.sync.dma_start(out=xT_sb[:, kk, :],
                                  in_=xT_d[kk * 128:(kk + 1) * 128, :])
            nc.sync.dma_start(out=cos_sb, in_=cos_d)
            nc.sync.dma_start(out=sin_sb, in_=sin_d)
            nc.sync.dma_start(out=perm_sb, in_=perm_d)
            make_identity(nc, ident_sb)

            def proj(w_sb, m, c):
                ps = pps.tile([128, 512], F32, tag="projps")
                for kk in range(KT):
                    nc.tensor.matmul(ps,
                                     w_sb[:, kk, m * 128:(m + 1) * 128],
                                     xT_sb[:, kk, c * 512:(c + 1) * 512],
                                     start=(kk == 0), stop=(kk == KT - 1))
                return ps

            def rope_store(raw, dst, bias_ap, c):
                rot = rps.tile([128, 512], F32, tag="rot")
                nc.tensor.matmul(rot, perm_sb, raw, start=True, stop=True)
                t1 = tmpp.tile([128, 512], F32, tag="t1")
                nc.vector.tensor_mul(t1, raw, cos_sb[:, c * 512:(c + 1) * 512])
                t2 = tmpp.tile([128, 512], F32, tag="t2")
                nc.vector.tensor_mul(t2, rot, sin_sb[:, c * 512:(c + 1) * 512])
                # dst = (t2 + bias) + t1
                nc.vector.scalar_tensor_tensor(dst, t2, bias_ap, t1,
                                               op0=ALU.add, op1=ALU.add)

            for c in range(NCH):
                for h in range(HPC):
                    ps = proj(wq_sb, h, c)
                    raw = rawp.tile([128, 512], BF, tag="qraw")
                    nc.scalar.copy(raw, ps)
                    rope_store(raw, qT_sb[:, h, c * 512:(c + 1) * 512],
                               bq_sb[:, h:h + 1], c)
                ps = proj(wk_sb, 0, c)
                raw = rawp.tile([128, 512], BF, tag="kraw")
                nc.scalar.copy(raw, ps)
                rope_store(raw, kT_sb[:, c * 512:(c + 1) * 512], bk_sb[:, 0:1], c)

                ps = proj(wv_sb, 0, c)
                vraw = rawp.tile([128, 512], F32, tag="vraw")
                nc.scalar.activation(vraw, ps, func=AF.Identity, bias=bv_sb[:, 0:1])
                for j in range(4):
                    t = c * 4 + j
                    tp = vtp.tile([128, 128], F32, tag="vtps")
                    nc.tensor.transpose(tp, vraw[:, j * 128:(j + 1) * 128], ident_sb)
                    nc.vector.tensor_copy(v_sb[:, t, :], tp)

        # ---------------- Phase B + C interleaved ---------------------------
        with tc.tile_pool(name="sps", bufs=3, space="PSUM") as sps, \
             tc.tile_pool(name="ops", bufs=2, space="PSUM") as opsp, \
             tc.tile_pool(name="smp", bufs=2, space="PSUM") as smp, \
             tc.tile_pool(name="cps", bufs=1, space="PSUM") as cps, \
             tc.tile_pool(name="expp", bufs=4) as expp, \
             tc.tile_pool(name="accp", bufs=2) as accp, \
             tc.tile_pool(name="rcp", bufs=2) as rcp, \
             tc.tile_pool(name="osbp", bufs=3) as osbp:

            for c in range(NCH):
                for h in range(HPC):
                    out_ps = opsp.tile([128, 512], F32, tag="outps")
                    acc = accp.tile([128, 512], F32, tag="acc")
                    nt = 4 * c + 4
                    for t in range(nt):
                        s_ps = sps.tile([128, 512], F32, tag="sps")
                        nc.tensor.matmul(s_ps,
                                         kT_sb[:, t * 128:(t + 1) * 128],
                                         qT_sb[:, h, c * 512:(c + 1) * 512],
                                         start=True, stop=True)
                        e = expp.tile([128, 512], BF, tag="exp")
                        nc.scalar.activation(e, s_ps, func=AF.Exp, scale=SCALE)
                        jb = t - 4 * c
                        if jb >= 0:
                            if jb > 0:
                                nc.vector.memset(e[:, 0:jb * 128], 0.0)
                            nc.vector.tensor_mul(e[:, jb * 128:(jb + 1) * 128],
                                                 e[:, jb * 128:(jb + 1) * 128],
                                                 mask_sb)
                        if t == 0:
                            nc.vector.tensor_copy(acc, e)
                        else:
                            nc.vector.tensor_add(acc, acc, e)
                        nc.tensor.matmul(out_ps, v_sb[:, t, :], e,
                                         start=(t == 0), stop=(t == nt - 1))
                    sums = smp.tile([1, 512], F32, tag="sums")
                    nc.tensor.matmul(sums, ones_sb, acc, start=True, stop=True)
                    recip = rcp.tile([1, 512], F32, tag="recip")
                    nc.vector.reciprocal(recip, sums)
                    rbc = rcp.tile([128, 512], F32, tag="rbc")
                    nc.gpsimd.partition_broadcast(rbc, recip)
                    nc.vector.tensor_mul(outT_sb[:, h, c * 512:(c + 1) * 512],
                                         out_ps, rbc)

                # phase C for the s-tiles of this chunk
                for m in range(4 * c, 4 * c + 4):
                    for n in range(NCH):
                        ps = cps.tile([128, 512], F32, tag="cpsum")
                        for h in range(HPC):
                            nc.tensor.matmul(ps,
                                             outT_sb[:, h, m * 128:(m + 1) * 128],
                                             wo_sb[:, h, n * 512:(n + 1) * 512],
                                             start=(h == 0), stop=(h == HPC - 1))
                        ob = osbp.tile([128, 512], F32, tag="osb")
                        nc.scalar.copy(ob, ps)
                        nc.sync.dma_start(
                            out=out_d[m * 128:(m + 1) * 128, n * 512:(n + 1) * 512],
                            in_=ob)

    nc.compile()
    return nc


def _get_program():
    global _PROGRAM
    if _PROGRAM is None:
        _PROGRAM = _build_program()
    return _PROGRAM


def _host_tables():
    bf16 = ml_dtypes.bfloat16
    pos = np.arange(S, dtype=np.float32)[:, None]
    i = np.arange(DH // 2, dtype=np.float32)
    omega = np.exp((-2.0 * i / DH * np.log(np.float32(1_000_000.0))).astype(np.float32))
    ang = (pos * omega).astype(np.float32)
    sinT = np.ascontiguousarray(np.repeat(np.sin(ang), 2, axis=-1).T)
    cosT = np.ascontiguousarray(np.repeat(np.cos(ang), 2, axis=-1).T)
    P = np.zeros((DH, DH), np.float32)
    for ii in range(DH // 2):
        P[2 * ii, 2 * ii + 1] = -1.0
        P[2 * ii + 1, 2 * ii] = 1.0
    permT = np.ascontiguousarray(P.T).astype(bf16)
    maskLT = np.triu(np.ones((128, 128), np.float32)).astype(bf16)
    return cosT, sinT, permT, maskLT


def _install_ntff_hook():
    """Optional: register the axon NTFF profiling hook (missing antenv.axon_hooks
    shim) so run_bass_kernel_spmd(trace=True) can capture HW exec time."""
    import types
    try:
        import antenv
        if 'antenv.axon_hooks' not in sys.modules:
            mod = types.ModuleType('antenv.axon_hooks')
            _hook = [None]
            mod.set_axon_ntff_profile_hook = lambda h: _hook.__setitem__(0, h)
            mod.get_axon_ntff_profile_hook = lambda: _hook[0]
            sys.modules['antenv.axon_hooks'] = mod
            antenv.axon_hooks = mod
        if '/root/.axon_site' not in sys.path:
            sys.path.insert(0, '/root/.axon_site')
        from trn_agent_boot.trn_boot import _ntff_profile_via_ctypes
        sys.modules['antenv.axon_hooks'].set_axon_ntff_profile_hook(
            _ntff_profile_via_ctypes('/opt/axon/libaxon_pjrt.so'))
        bass_utils.upload_artifacts = lambda tmpdir: tmpdir
        return True
    except Exception:
        return False


def kernel(x, wq, bq, wk, bk, wv, bv, wo, bo, masked=None, **_unused):
    global LAST_RESULTS
    bf16 = ml_dtypes.bfloat16
    nc = _get_program()

    x = np.asarray(x, np.float32)
    wq = np.asarray(wq, np.float32)
    wk = np.asarray(wk, np.float32)
    wv = np.asarray(wv, np.float32)
    wo = np.asarray(wo, np.float32)
    bq = np.asarray(bq, np.float32)
    bk = np.asarray(bk, np.float32)
    bv = np.asarray(bv, np.float32)
    bo = np.asarray(bo, np.float32)

    cosT, sinT, permT, maskLT = _host_tables()

    xT = [np.ascontiguousarray(x[b].T).astype(bf16) for b in range(B)]
    in_maps = []
    for core in range(N_CORES):
        b, g = divmod(core, G)
        cs = slice(g * 512, (g + 1) * 512)          # q-channel / out-channel slice
        ks = slice(g * 128, (g + 1) * 128)          # kv-channel slice
        in_maps.append({
            "xT": xT[b],
            "wqT": np.ascontiguousarray(wq[cs, :].T).astype(bf16),
            "wkT": np.ascontiguousarray(wk[ks, :].T).astype(bf16),
            "wvT": np.ascontiguousarray(wv[ks, :].T).astype(bf16),
            "woT": np.ascontiguousarray(wo[:, cs].T).astype(bf16),
            "cosT": cosT,
            "sinT": sinT,
            "permT": permT,
            "maskLT": maskLT,
            "bq": np.ascontiguousarray(bq[cs].reshape(HPC, DH).T),
            "bk": np.ascontiguousarray(bk[ks].reshape(DH, 1)),
            "bv": np.ascontiguousarray(bv[ks].reshape(DH, 1)),
        })

    trace = bool(TRACE)
    if trace:
        trace = _install_ntff_hook()
    res = bass_utils.run_bass_kernel_spmd(nc, in_maps,
                                          core_ids=list(range(N_CORES)),
                                          trace=trace)
    LAST_RESULTS = res

    out = np.zeros((B, S, D), np.float32)
    for core in range(N_CORES):
        b = core // G
        out[b] += res.results[core]["part"]
    out += bo[None, None, :]
    return out


# revision 7
# speedup vs baseline: 1.1223x; 1.1223x over previous
"""Trainium2 Bass kernel for GQA attention (B=2, S=2048, D=2048, H=16, G=4 kv-heads,
DH=128) with interleaved RoPE (base 1e6) and causal mask.

Sharding: one (batch b, kv-group g) pair per NeuronCore -> 8 cores. Each core
computes its 4 q-heads against its single kv-head (Megatron-style column-split
of w_q/w_k/w_v, row-split of w_o) and produces a partial (S, D) output-projection
product; the host sums the 4 partials per batch and adds bo.

Device dataflow per core (all matmuls bf16 with f32 PSUM accumulate):
  A) qT/kT/vT = W^T-slices @ x^T (transposed projections, dmodel contraction),
     interleaved RoPE applied in the transposed layout via a +-1 permutation
     matmul plus two DVE multiplies with host-provided cos/sin tables;
     v transposed back to [sk, dh] via PE transpose.
  B) per (head, sq-chunk of 512): scoresT tiles [sk=128, sq=512] via PE,
     exp on ScalarE (scale=1/sqrt(128)) straight out of PSUM -> bf16 attn
     weights, causal masking on diagonal tiles, DVE accumulates exp sums,
     PV matmul accumulates out^T [dh, sq] in PSUM over sk tiles; column sums
     via a ones-vector matmul, reciprocal + partition-broadcast, fused
     normalize-and-evict on DVE.
  C) partial = out_heads^T^T @ wo^T-slice, streamed to DRAM as f32.
"""
import sys
import os

if '/opt/trn_rl_repo' not in sys.path:
    sys.path.insert(0, '/opt/trn_rl_repo')

import numpy as np
import ml_dtypes

from contextlib import ExitStack

import concourse.bass as bass
import concourse.mybir as mybir
import concourse.tile as tile
from concourse import bacc
import concourse.bass_utils as bass_utils
from concourse.masks import make_identity

BF = mybir.dt.bfloat16
F32 = mybir.dt.float32
AF = mybir.ActivationFunctionType
ALU = mybir.AluOpType

B, S, D, H, G = 2, 2048, 2048, 16, 4
DH = 128
HPC = H // G          # q heads per core
KT = D // 128         # dmodel k-tiles
NCH = S // 512        # sq chunks
SCALE = float(1.0 / np.sqrt(DH))
N_CORES = 8

TRACE = False          # set by test harness to capture an NTFF profile
LAST_RESULTS = None    # BassKernelResults of the most recent run (for test.py)

_PROGRAM = None


def _build_program():
    nc = bacc.Bacc("TRN2", target_bir_lowering=False, debug=False,
                   num_devices=N_CORES)

    def din(name, shape, dtype=BF):
        return nc.dram_tensor(name, shape, dtype, kind="ExternalInput").ap()

    xT_d = din("xT", [D, S])
    wq_d = din("wqT", [D, 512])
    wk_d = din("wkT", [D, DH])
    wv_d = din("wvT", [D, DH])
    wo_d = din("woT", [512, D])
    cos_d = din("cosT", [DH, S], F32)
    sin_d = din("sinT", [DH, S], F32)
    perm_d = din("permT", [DH, DH])
    mask_d = din("maskLT", [DH, DH])
    bq_d = din("bq", [DH, HPC], F32)
    bk_d = din("bk", [DH, 1], F32)
    bv_d = din("bv", [DH, 1], F32)
    out_d = nc.dram_tensor("part", [S, D], F32, kind="ExternalOutput").ap()

    with tile.TileContext(nc) as tc, ExitStack() as ctx:
        consts = ctx.enter_context(tc.tile_pool(name="consts", bufs=1))

        # persistent SBUF tensors
        wq_sb = consts.tile([128, KT, 512], BF, tag="wq")
        wk_sb = consts.tile([128, KT, DH], BF, tag="wk")
        wv_sb = consts.tile([128, KT, DH], BF, tag="wv")
        wo_sb = consts.tile([128, HPC, D], BF, tag="wo")
        mask_sb = consts.tile([128, 128], BF, tag="mask")
        bq_sb = consts.tile([128, HPC], F32, tag="bq")
        bk_sb = consts.tile([128, 1], F32, tag="bk")
        bv_sb = consts.tile([128, 1], F32, tag="bv")
        ones_sb = consts.tile([128, 1], BF, tag="ones")
        qT_sb = consts.tile([128, HPC, S], BF, tag="qT")
        kT_sb = consts.tile([128, S], BF, tag="kT")
        v_sb = consts.tile([128, KT, DH], BF, tag="v")
        outT_sb = consts.tile([128, HPC, S], BF, tag="outT")

        # ---------------- Phase A: projections + RoPE + v transpose ----------
        with tc.tile_pool(name="aconst", bufs=1) as aconst, \
             tc.tile_pool(name="pps", bufs=4, space="PSUM") as pps, \
             tc.tile_pool(name="rps", bufs=2, space="PSUM") as rps, \
             tc.tile_pool(name="vtp", bufs=2, space="PSUM") as vtp, \
             tc.tile_pool(name="rawp", bufs=3) as rawp, \
             tc.tile_pool(name="tmpp", bufs=3) as tmpp:

            xT_sb = aconst.tile([128, KT, S], BF, tag="xT")
            cos_sb = aconst.tile([128, S], F32, tag="cos")
            sin_sb = aconst.tile([128, S], F32, tag="sin")
            perm_sb = aconst.tile([128, 128], BF, tag="perm")
            ident_sb = aconst.tile([128, 128], F32, tag="ident")

            # DMA in consumption order: x/w tiles interleaved, then rope
            # tables; wo/mask (phase B/C) last.
            for kk in range(KT):
                nc.sync.dma_start(out=xT_sb[:, kk, :],
                                  in_=xT_d[kk * 128:(kk + 1) * 128, :])
                nc.sync.dma_start(out=wq_sb[:, kk, :],
                                  in_=wq_d[kk * 128:(kk + 1) * 128, :])
                nc.sync.dma_start(out=wk_sb[:, kk, :],
                                  in_=wk_d[kk * 128:(kk + 1) * 128, :])
                nc.sync.dma_start(out=wv_sb[:, kk, :],
                                  in_=wv_d[kk * 128:(kk + 1) * 128, :])
            nc.sync.dma_start(out=cos_sb, in_=cos_d)
            nc.sync.dma_start(out=sin_sb, in_=sin_d)
            nc.sync.dma_start(out=perm_sb, in_=perm_d)
            nc.sync.dma_start(out=bq_sb, in_=bq_d)
            nc.sync.dma_start(out=bk_sb, in_=bk_d)
            nc.sync.dma_start(out=bv_sb, in_=bv_d)
            nc.sync.dma_start(out=mask_sb, in_=mask_d)
            for h in range(HPC):
                nc.sync.dma_start(out=wo_sb[:, h, :],
                                  in_=wo_d[h * 128:(h + 1) * 128, :])
            nc.vector.memset(ones_sb, 1.0)
            make_identity(nc, ident_sb)

            def rope_store(raw, dst, bias_ap, c):
                rot = rps.tile([128, 512], F32, tag="rot")
                nc.tensor.matmul(rot, perm_sb, raw, start=True, stop=True)
                t1 = tmpp.tile([128, 512], F32, tag="t1")
                nc.vector.tensor_mul(t1, raw, cos_sb[:, c * 512:(c + 1) * 512])
                t2 = tmpp.tile([128, 512], F32, tag="t2")
                nc.vector.tensor_mul(t2, rot, sin_sb[:, c * 512:(c + 1) * 512])
                # dst = (t2 + bias) + t1
                nc.vector.scalar_tensor_tensor(dst, t2, bias_ap, t1,
                                               op0=ALU.add, op1=ALU.add)

            # m-tiles: 4 q heads, then k, then v. For each m, hold the
            # stationary weight tile across all 4 sq-chunks (LDW amortized 4x,
            # 4 concurrent PSUM accumulation banks).
            for mi in range(6):
                pss = [pps.tile([128, 512], F32, tag="projps", name=f"projps{_c}")
                       for _c in range(NCH)]
                w_sb, mcol = (wq_sb, mi) if mi < 4 else \
                             ((wk_sb, 0) if mi == 4 else (wv_sb, 0))
                for kk in range(KT):
                    for c in range(NCH):
                        nc.tensor.matmul(pss[c],
                                         w_sb[:, kk, mcol * 128:(mcol + 1) * 128],
                                         xT_sb[:, kk, c * 512:(c + 1) * 512],
                                         start=(kk == 0), stop=(kk == KT - 1))
                for c in range(NCH):
                    if mi < 4:
                        raw = rawp.tile([128, 512], BF, tag="qraw")
                        nc.scalar.copy(raw, pss[c])
                        rope_store(raw, qT_sb[:, mi, c * 512:(c + 1) * 512],
                                   bq_sb[:, mi:mi + 1], c)
                    elif mi == 4:
                        raw = rawp.tile([128, 512], BF, tag="kraw")
                        nc.scalar.copy(raw, pss[c])
                        rope_store(raw, kT_sb[:, c * 512:(c + 1) * 512],
                                   bk_sb[:, 0:1], c)
                    else:
                        vraw = rawp.tile([128, 512], F32, tag="vraw")
                        nc.scalar.activation(vraw, pss[c], func=AF.Identity,
                                             bias=bv_sb[:, 0:1])
                        for j in range(4):
                            t = c * 4 + j
                            tp = vtp.tile([128, 128], F32, tag="vtps")
                            nc.tensor.transpose(tp, vraw[:, j * 128:(j + 1) * 128],
                                                ident_sb)
                            nc.vector.tensor_copy(v_sb[:, t, :], tp)

        # ---------------- Phase B + C interleaved ---------------------------
        # Per (chunk c, h-pair): scores/PV share each stationary (kT[t], v[t])
        # across the two heads; exp sums accumulate on PE via ones-matmuls
        # into a per-head PSUM row. Diagonal tiles only compute the
        # non-masked column range.
        with tc.tile_pool(name="sps", bufs=2, space="PSUM") as sps, \
             tc.tile_pool(name="ops", bufs=2, space="PSUM") as opsp, \
             tc.tile_pool(name="smp", bufs=2, space="PSUM") as smp, \
             tc.tile_pool(name="cps", bufs=2, space="PSUM") as cps, \
             tc.tile_pool(name="expp", bufs=4) as expp, \
             tc.tile_pool(name="rcp", bufs=2) as rcp, \
             tc.tile_pool(name="osbp", bufs=3) as osbp:

            def emit_c_group(m, np_):
                ns = (2 * np_, 2 * np_ + 1)
                pso = {n: cps.tile([128, 512], F32, tag="cpsum", name=f"cpsum{n}")
                       for n in ns}
                for h in range(HPC):
                    for n in ns:
                        nc.tensor.matmul(pso[n],
                                         outT_sb[:, h, m * 128:(m + 1) * 128],
                                         wo_sb[:, h, n * 512:(n + 1) * 512],
                                         start=(h == 0), stop=(h == HPC - 1))
                for n in ns:
                    ob = osbp.tile([128, 512], F32, tag="osb")
                    nc.vector.tensor_copy(ob, pso[n])
                    nc.sync.dma_start(
                        out=out_d[m * 128:(m + 1) * 128, n * 512:(n + 1) * 512],
                        in_=ob)

            for c in range(NCH):
                nt = 4 * c + 4
                for hp in range(HPC // 2):
                    hs = (2 * hp, 2 * hp + 1)
                    out_ps = {h: opsp.tile([128, 512], F32, tag="outps", name=f"outps{h}")
                              for h in hs}
                    sums = {h: smp.tile([1, 512], F32, tag="sums", name=f"sums{h}")
                            for h in hs}
                    for t in range(nt):
                        jb = t - 4 * c
                        off = max(jb, 0) * 128   # first valid sq column
                        w = 512 - off
                        cl, ch_ = c * 512 + off, (c + 1) * 512
                        es = {}
                        for h in hs:
                            s_ps = sps.tile([128, 512], F32, tag="sps")
                            nc.tensor.matmul(s_ps[:, off:],
                                             kT_sb[:, t * 128:(t + 1) * 128],
                                             qT_sb[:, h, cl:ch_],
                                             start=True, stop=True)
                            e = expp.tile([128, 512], BF, tag="exp")
                            nc.scalar.activation(e[:, off:], s_ps[:, off:],
                                                 func=AF.Exp, scale=SCALE)
                            if jb >= 0:
                                nc.vector.tensor_mul(e[:, off:off + 128],
                                                     e[:, off:off + 128], mask_sb)
                            es[h] = e
                        for h in hs:
                            nc.tensor.matmul(sums[h][:, off:], ones_sb,
                                             es[h][:, off:],
                                             start=(t == 0), stop=(t == nt - 1))
                        for h in hs:
                            nc.tensor.matmul(out_ps[h][:, off:], v_sb[:, t, :],
                                             es[h][:, off:],
                                             start=(t == 0), stop=(t == nt - 1))
                    for h in hs:
                        recip = rcp.tile([1, 512], F32, tag="recip")
                        nc.vector.reciprocal(recip, sums[h])
                        rbc = rcp.tile([128, 512], F32, tag="rbc")
                        nc.gpsimd.partition_broadcast(rbc, recip)
                        nc.vector.tensor_mul(outT_sb[:, h, c * 512:(c + 1) * 512],
                                             out_ps[h], rbc)

                    # fill the pair-boundary with phase-C work of the
                    # previous chunk (its outT rows are complete)
                    if c > 0:
                        for m in range(4 * (c - 1) + 2 * hp,
                                       4 * (c - 1) + 2 * hp + 2):
                            for np_ in range(NCH // 2):
                                emit_c_group(m, np_)

            # phase C for the final chunk
            for m in range(4 * (NCH - 1), 4 * NCH):
                for np_ in range(NCH // 2):
                    emit_c_group(m, np_)

    nc.compile()
    return nc


def _get_program():
    global _PROGRAM
    if _PROGRAM is None:
        _PROGRAM = _build_program()
    return _PROGRAM


def _host_tables():
    bf16 = ml_dtypes.bfloat16
    pos = np.arange(S, dtype=np.float32)[:, None]
    i = np.arange(DH // 2, dtype=np.float32)
    omega = np.exp((-2.0 * i / DH * np.log(np.float32(1_000_000.0))).astype(np.float32))
    ang = (pos * omega).astype(np.float32)
    sinT = np.ascontiguousarray(np.repeat(np.sin(ang), 2, axis=-1).T)
    cosT = np.ascontiguousarray(np.repeat(np.cos(ang), 2, axis=-1).T)
    P = np.zeros((DH, DH), np.float32)
    for ii in range(DH // 2):
        P[2 * ii, 2 * ii + 1] = -1.0
        P[2 * ii + 1, 2 * ii] = 1.0
    permT = np.ascontiguousarray(P.T).astype(bf16)
    maskLT = np.triu(np.ones((128, 128), np.float32)).astype(bf16)
    return cosT, sinT, permT, maskLT


def _install_ntff_hook():
    """Optional: register the axon NTFF profiling hook (missing antenv.axon_hooks
    shim) so run_bass_kernel_spmd(trace=True) can capture HW exec time."""
    import types
    try:
        import antenv
        if 'antenv.axon_hooks' not in sys.modules:
            mod = types.ModuleType('antenv.axon_hooks')
            _hook = [None]
            mod.set_axon_ntff_profile_hook = lambda h: _hook.__setitem__(0, h)
            mod.get_axon_ntff_profile_hook = lambda: _hook[0]
            sys.modules['antenv.axon_hooks'] = mod
            antenv.axon_hooks = mod
        if '/root/.axon_site' not in sys.path:
            sys.path.insert(0, '/root/.axon_site')
        from trn_agent_boot.trn_boot import _ntff_profile_via_ctypes
        sys.modules['antenv.axon_hooks'].set_axon_ntff_profile_hook(
            _ntff_profile_via_ctypes('/opt/axon/libaxon_pjrt.so'))
        bass_utils.upload_artifacts = lambda tmpdir: tmpdir
        return True
    except Exception:
        return False


def kernel(x, wq, bq, wk, bk, wv, bv, wo, bo, masked=None, **_unused):
    global LAST_RESULTS
    bf16 = ml_dtypes.bfloat16
    nc = _get_program()

    x = np.asarray(x, np.float32)
    wq = np.asarray(wq, np.float32)
    wk = np.asarray(wk, np.float32)
    wv = np.asarray(wv, np.float32)
    wo = np.asarray(wo, np.float32)
    bq = np.asarray(bq, np.float32)
    bk = np.asarray(bk, np.float32)
    bv = np.asarray(bv, np.float32)
    bo = np.asarray(bo, np.float32)

    cosT, sinT, permT, maskLT = _host_tables()

    xT = [np.ascontiguousarray(x[b].T).astype(bf16) for b in range(B)]
    in_maps = []
    for core in range(N_CORES):
        b, g = divmod(core, G)
        cs = slice(g * 512, (g + 1) * 512)          # q-channel / out-channel slice
        ks = slice(g * 128, (g + 1) * 128)          # kv-channel slice
        in_maps.append({
            "xT": xT[b],
            "wqT": np.ascontiguousarray(wq[cs, :].T).astype(bf16),
            "wkT": np.ascontiguousarray(wk[ks, :].T).astype(bf16),
            "wvT": np.ascontiguousarray(wv[ks, :].T).astype(bf16),
            "woT": np.ascontiguousarray(wo[:, cs].T).astype(bf16),
            "cosT": cosT,
            "sinT": sinT,
            "permT": permT,
            "maskLT": maskLT,
            "bq": np.ascontiguousarray(bq[cs].reshape(HPC, DH).T),
            "bk": np.ascontiguousarray(bk[ks].reshape(DH, 1)),
            "bv": np.ascontiguousarray(bv[ks].reshape(DH, 1)),
        })

    trace = bool(TRACE)
    if trace:
        trace = _install_ntff_hook()
    res = bass_utils.run_bass_kernel_spmd(nc, in_maps,
                                          core_ids=list(range(N_CORES)),
                                          trace=trace)
    LAST_RESULTS = res

    out = np.zeros((B, S, D), np.float32)
    for core in range(N_CORES):
        b = core // G
        out[b] += res.results[core]["part"]
    out += bo[None, None, :]
    return out
